# revision 1
# baseline (speedup 1.0000x reference)
"""Upfirdn2d-style blur kernel for Trainium2 (Bass/Tile), 8-core SPMD.

Computes: zero-insertion 2x upsample + pad(2,1,2,1) + depthwise 4x4 FIR
  filter outer([1,3,3,1],[1,3,3,1])/64 * 4  (separable, symmetric)
on x of shape (16, 512, 32, 32) f32 -> (16, 512, 64, 64) f32.

Polyphase separable decomposition (verified vs reference, ~1e-7 abs err):
  vertical  : t[2r]   = (3*x[r] + x[r-1])/16 ; t[2r+1] = (3*x[r] + x[r+1])/16
  horizontal: o[2c]   = 3*t[c] + t[c-1]      ; o[2c+1] = 3*t[c] + t[c+1]
(out-of-range x/t taps are zero)

Sharding: pure data parallel over the 8192 independent images (batch*channel,
conv is depthwise) -> 1024 images per core, no cross-core communication.

Per-core layout: 32x32 images on SBUF partitions, 2 images per partition per
iteration (4 iterations x 256 images). Each 2-tap polyphase combine is ONE
fused DVE instruction (scalar_tensor_tensor: out = (in0 * scalar) + in1).

walrus here accepts only ONE sync-wait command per instruction, so the
program is structured so every instruction needs at most one new semaphore
observation:
  - exactly 8 DMAs total (4 in + 4 out) = the 8 DMAHW sem lanes, so no
    lane-reuse waits;
  - input DMAs use bufs=N_ITERS (no slot reuse -> no WAR/WAW waits);
  - all compute on DVE so compute deps fold into the single DVE sem;
  - a 1-element "wait absorber" op between the vertical and horizontal
    passes so the first o-writer carries only the DMA-out WAR wait.
"""

import numpy as np

import concourse.bass as bass
import concourse.mybir as mybir
import concourse.tile as tile
from concourse.bass_utils import run_bass_kernel_spmd

N_CORES = 8
B, C, H, W = 16, 512, 32, 32
IMGS = B * C                  # 8192 independent images
PER_CORE = IMGS // N_CORES    # 1024
P = 128                       # SBUF partitions
SUB = 2                       # images per partition per iteration
N_ITERS = PER_CORE // (P * SUB)   # 4
IMG = H * W                   # 1024 elems per input image
OIMG = 4 * IMG                # 4096 elems per output image

F32 = mybir.dt.float32
A = mybir.AluOpType


def _split_multi_waits(nc: bass.Bass) -> None:
    """walrus rejects >1 sync-wait per instruction; hoist extras onto NoOps.

    A NoOp on the same engine queue immediately before the instruction
    executes its wait first, so splitting the AND-list of waits across a
    NoOp chain is semantically identical.
    """
    for fn in nc.m.functions:
        for bb in fn.blocks:
            insts = bb.instructions
            i = 0
            while i < len(insts):
                inst = insts[i]
                si = inst.sync_info
                if si is not None and len(si.on_wait) > 1:
                    waits = list(si.on_wait)
                    for j, w in enumerate(waits[:-1]):
                        nop = mybir.InstNoOp(
                            name=nc.get_next_instruction_name(),
                            text_hint=f"wait_split_{j}")
                        nop.engine = inst.engine
                        nop.sync_info = mybir.SyncInfo(
                            on_wait=[w], on_update=[])
                        insts.insert(i, nop)
                        i += 1
                    inst.sync_info = mybir.SyncInfo(
                        on_wait=[waits[-1]], on_update=list(si.on_update))
                i += 1


def build_nc(split_waits: bool = True, repeat: int = 1,
             v_eng: str = "vector", sc_eng: str = "scalar",
             in_q: str = "scalar", out_q: str = "sync",
             sub: int = SUB, po_bufs: int = 2, pt_bufs: int = 2,
             pxq_bufs: int = 2, bnd_eng: str = "scalar") -> bass.Bass:
    """Engine map (defaults): input DMA on ACT HWDGE queue, output DMA on SP
    queue (so outputs never block input prefetch), vertical STTs on GPSIMD,
    horizontal STTs on DVE, scalings/boundary taps on ACT. Multi-wait
    instructions are legalized by _split_multi_waits."""
    nc = bass.Bass()
    x = nc.dram_tensor("x", (PER_CORE, IMG), F32, kind="ExternalInput")
    out = nc.dram_tensor("out", (PER_CORE, OIMG), F32, kind="ExternalOutput")
    ev = getattr(nc, v_eng)      # vertical-pass STT engine
    sc = getattr(nc, sc_eng)     # pre-scale engine
    bd = getattr(nc, bnd_eng)    # boundary-tap engine
    in_dma = getattr(nc, in_q)
    out_dma = getattr(nc, out_q)

    n_iters = PER_CORE // (P * sub)
    with tile.TileContext(nc) as tc:
        with (
            tc.tile_pool(name="pin", bufs=n_iters) as pin,
            tc.tile_pool(name="pxq", bufs=pxq_bufs) as pxq,
            tc.tile_pool(name="pt", bufs=pt_bufs) as pt,
            tc.tile_pool(name="po", bufs=po_bufs) as po,
        ):
            for it in range(repeat * n_iters):
                i = it % n_iters
                base = i * P * sub

                xin = pin.tile([P, sub * IMG], F32, tag="xin")
                # partition p holds images base+p and base+P+p
                x_dram = bass.AP(x, base * IMG,
                                 [[IMG, P], [P * IMG, sub], [1, IMG]])
                xin_v = xin[:].rearrange("p (s c) -> p s c", s=sub)
                in_dma.dma_start(out=xin_v, in_=x_dram)

                xq = pxq.tile([P, sub * IMG], F32, tag="xq")
                t = pt.tile([P, sub * 2 * IMG], F32, tag="t")
                o = po.tile([P, sub * OIMG], F32, tag="o")

                # xq = x/16 — second-tap operand for the vertical pass
                for h in range(sub):
                    sc.mul(xq[:, h * IMG:(h + 1) * IMG],
                           xin[:, h * IMG:(h + 1) * IMG], 1.0 / 16.0)

                for h in range(sub):
                    x3 = xin[:, h * IMG:(h + 1) * IMG].rearrange(
                        "p (r c) -> p r c", c=W)
                    q3 = xq[:, h * IMG:(h + 1) * IMG].rearrange(
                        "p (r c) -> p r c", c=W)
                    th = t[:, h * 2 * IMG:(h + 1) * 2 * IMG]
                    t3 = th.rearrange("p (r c) -> p r c", c=W)
                    # t[2r] = (x[r]*3/16) + x[r-1]/16, r=1..31
                    ev.scalar_tensor_tensor(
                        t3[:, 2::2, :], x3[:, 1:, :], 3.0 / 16.0,
                        q3[:, :31, :], A.mult, A.add)
                    # t[2r+1] = (x[r]*3/16) + x[r+1]/16, r=0..30
                    ev.scalar_tensor_tensor(
                        t3[:, 1:62:2, :], x3[:, :31, :], 3.0 / 16.0,
                        q3[:, 1:, :], A.mult, A.add)
                    # boundary rows {0,63} <- (3/16) * x rows {0,31}
                    t_bnd = bass.AP(th.tensor, th.offset,
                                    [th.ap[0], [63 * W, 2], [1, W]])
                    x_bnd = bass.AP(xin[:].tensor,
                                    xin[:].offset + h * IMG,
                                    [xin[:].ap[0], [31 * W, 2], [1, W]])
                    if bnd_eng == "scalar":
                        bd.mul(t_bnd, x_bnd, 3.0 / 16.0)
                    else:
                        bd.tensor_scalar_mul(t_bnd, x_bnd, 3.0 / 16.0)

                for h in range(sub):
                    th = t[:, h * 2 * IMG:(h + 1) * 2 * IMG]
                    oh = o[:, h * OIMG:(h + 1) * OIMG]
                    t3 = th.rearrange("p (r c) -> p r c", c=W)
                    o3 = oh.rearrange("p (r c) -> p r c", c=2 * W)
                    # o[2c] = (t[c]*3) + t[c-1], c=1..31
                    nc.vector.scalar_tensor_tensor(
                        o3[:, :, 2::2], t3[:, :, 1:], 3.0, t3[:, :, :31],
                        A.mult, A.add)
                    # o[2c+1] = (t[c]*3) + t[c+1], c=0..30
                    nc.vector.scalar_tensor_tensor(
                        o3[:, :, 1:62:2], t3[:, :, :31], 3.0, t3[:, :, 1:],
                        A.mult, A.add)
                    # boundary cols {0,63} <- 3 * t cols {0,31}
                    o_bnd = bass.AP(oh.tensor, oh.offset,
                                    [oh.ap[0], [2 * W, 2 * H], [63, 2]])
                    t_bnd2 = bass.AP(th.tensor, th.offset,
                                     [th.ap[0], [W, 2 * H], [31, 2]])
                    if bnd_eng == "scalar":
                        bd.mul(o_bnd, t_bnd2, 3.0)
                    else:
                        bd.tensor_scalar_mul(o_bnd, t_bnd2, 3.0)

                o_dram = bass.AP(out, base * OIMG,
                                 [[OIMG, P], [P * OIMG, sub], [1, OIMG]])
                o_v = o[:].rearrange("p (s c) -> p s c", s=sub)
                out_dma.dma_start(out=o_dram, in_=o_v)
    if split_waits:
        _split_multi_waits(nc)
    return nc


def kernel(x: np.ndarray) -> np.ndarray:
    x = np.ascontiguousarray(np.asarray(x), dtype=np.float32)
    assert x.shape == (B, C, H, W), x.shape
    flat = x.reshape(IMGS, IMG)
    in_maps = [
        {"x": flat[c * PER_CORE:(c + 1) * PER_CORE]} for c in range(N_CORES)
    ]
    nc = build_nc()
    res = run_bass_kernel_spmd(nc, in_maps, core_ids=list(range(N_CORES)))
    outs = [res.results[c]["out"] for c in range(N_CORES)]
    full = np.concatenate(outs, axis=0).reshape(B, C, 2 * H, 2 * W)
    return full


if __name__ == "__main__":
    rng = np.random.default_rng(0)
    xt = rng.standard_normal((B, C, H, W), dtype=np.float32)
    yt = kernel(xt)
    print("out", yt.shape, yt.dtype)



# revision 2
# speedup vs baseline: 1.9573x; 1.9573x over previous
"""Upfirdn2d blur kernel v2/v3: finer-grained pipeline for DMA saturation.

Math (polyphase separable 2x upsample + [1,3,3,1]^2/64*4 depthwise blur):
  vertical  : t[2r]   = (3*x[r] + x[r-1])/16 ; t[2r+1] = (3*x[r] + x[r+1])/16
  horizontal: o[2c]   = 3*t[c] + t[c-1]      ; o[2c+1] = 3*t[c] + t[c+1]
(out-of-range taps zero; boundary rows/cols keep only the center tap)

Sharding: pure data parallel over 8192 independent images -> 1024 per core.

v2 layout: 8 iterations x 128 images (1 image per partition). All 8 input
DMAs prefetch up front (pin bufs=8); per-iteration output DMA, optionally
split in row halves (h_split=2).

mode="f32": vertical + horizontal both f32 STT on DVE (1 elem/cycle).
mode="ttv": vertical as bf16 tensor_tensor (2x DVE mode) on pre-scaled
  bf16 operands x3=3x/16, xq=x/16 (ACT); t boundary rows are copies of x3
  rows {0,31}; horizontal reads bf16 t, writes f32 interleaved (1x STT).
  DVE per-iter drops ~4.3us -> ~3.1us+2.1us; rel err ~4e-3 << 2e-2 gate.
"""

import numpy as np

import concourse.bass as bass
import concourse.mybir as mybir
import concourse.tile as tile
from concourse.bass_utils import run_bass_kernel_spmd

N_CORES = 8
B, C, H, W = 16, 512, 32, 32
IMGS = B * C                  # 8192 independent images
PER_CORE = IMGS // N_CORES    # 1024
P = 128                       # SBUF partitions
N_ITERS = PER_CORE // P       # 8
IMG = H * W                   # 1024 elems per input image
OIMG = 4 * IMG                # 4096 elems per output image
OH, OW = 2 * H, 2 * W

F32 = mybir.dt.float32
BF16 = mybir.dt.bfloat16
A = mybir.AluOpType


def _split_multi_waits(nc: bass.Bass) -> None:
    """walrus rejects >1 sync-wait per instruction; hoist extras onto NoOps."""
    for fn in nc.m.functions:
        for bb in fn.blocks:
            insts = bb.instructions
            i = 0
            while i < len(insts):
                inst = insts[i]
                si = inst.sync_info
                if si is not None and len(si.on_wait) > 1:
                    waits = list(si.on_wait)
                    for j, w in enumerate(waits[:-1]):
                        nop = mybir.InstNoOp(
                            name=nc.get_next_instruction_name(),
                            text_hint=f"wait_split_{j}")
                        nop.engine = inst.engine
                        nop.sync_info = mybir.SyncInfo(
                            on_wait=[w], on_update=[])
                        insts.insert(i, nop)
                        i += 1
                    inst.sync_info = mybir.SyncInfo(
                        on_wait=[waits[-1]], on_update=list(si.on_update))
                i += 1


def build_nc(split_waits: bool = True, loop_repeat: int = 1,
             mode: str = "ttv",
             in_q: str = "sync", out_q: str = "sync",
             pin_bufs: int = N_ITERS, pxq_bufs: int = 2, pt_bufs: int = 2,
             po_bufs: int = 3, h_split: int = 1,
             body_reps: int = 1, timing_mode: bool = False,
             alt_oq: int = 0) -> bass.Bass:
    nc = bass.Bass()
    if timing_mode:
        # device-side scratch I/O: identical DMA/compute structure, but no
        # tunnel transfer of the 20MB payload per call -> low-noise slope
        x = nc.dram_tensor("x", (PER_CORE, IMG), F32, kind="Internal")
        out = nc.dram_tensor("out", (PER_CORE, OIMG), F32, kind="Internal")
        tin = nc.dram_tensor("tin", (1, 1), F32, kind="ExternalInput")
        tout = nc.dram_tensor("tout", (1, 1), F32, kind="ExternalOutput")
    else:
        x = nc.dram_tensor("x", (PER_CORE, IMG), F32, kind="ExternalInput")
        out = nc.dram_tensor("out", (PER_CORE, OIMG), F32,
                             kind="ExternalOutput")
    in_dma = getattr(nc, in_q)
    out_dma = getattr(nc, out_q)
    tdt = BF16 if mode == "ttv" else F32

    def body(tc, pin, pxq, pxb, pt, po):
        for i in range(N_ITERS):
            base = i * P

            xin = pin.tile([P, IMG], F32, tag="xin")
            x_dram = bass.AP(x, base * IMG, [[IMG, P], [1, IMG]])
            # prefetch: schedule all input DMA issues at the program head so
            # they never queue behind compute-dependent waits
            with tc.high_priority():
                in_dma.dma_start(out=xin[:], in_=x_dram)

            xq = pxq.tile([P, IMG], tdt, tag="xq")
            t = pt.tile([P, 2 * IMG], tdt, tag="t")
            o = po.tile([P, OIMG], F32, tag="o")

            t3 = t[:].rearrange("p (r c) -> p r c", c=W)
            q3 = xq[:].rearrange("p (r c) -> p r c", c=W)

            if mode == "ttv":
                # ACT pre-scales to bf16: x3 = 3x/16 (center), xq = x/16
                x3b = pxb.tile([P, IMG], BF16, tag="x3b")
                nc.scalar.mul(x3b[:], xin[:], 3.0 / 16.0)
                nc.scalar.mul(xq[:], xin[:], 1.0 / 16.0)
                x3 = x3b[:].rearrange("p (r c) -> p r c", c=W)
                # boundary rows of t are pure center taps: t[0]=x3[0],
                # t[63]=x3[31] (ACT copy, one 2-run AP)
                t_bnd = bass.AP(t[:].tensor, t[:].offset,
                                [t[:].ap[0], [(2 * H - 1) * W, 2], [1, W]])
                x3_bnd = bass.AP(x3b[:].tensor, x3b[:].offset,
                                 [x3b[:].ap[0], [(H - 1) * W, 2], [1, W]])
                nc.scalar.copy(t_bnd, x3_bnd)
                # t[2r] = x3[r] + xq[r-1], r=1..31 (bf16 TT -> 2x mode)
                nc.vector.tensor_tensor(
                    t3[:, 2::2, :], x3[:, 1:, :], q3[:, :H - 1, :], A.add)
                # t[2r+1] = x3[r] + xq[r+1], r=0..30
                nc.vector.tensor_tensor(
                    t3[:, 1:2 * H - 2:2, :], x3[:, :H - 1, :], q3[:, 1:, :],
                    A.add)
            else:
                # xq = x/16 — second-tap operand for the vertical pass
                nc.scalar.mul(xq[:], xin[:], 1.0 / 16.0)
                x3 = xin[:].rearrange("p (r c) -> p r c", c=W)
                # t[2r] = (x[r]*3/16) + x[r-1]/16, r=1..31
                nc.vector.scalar_tensor_tensor(
                    t3[:, 2::2, :], x3[:, 1:, :], 3.0 / 16.0,
                    q3[:, :H - 1, :], A.mult, A.add)
                # t[2r+1] = (x[r]*3/16) + x[r+1]/16, r=0..30
                nc.vector.scalar_tensor_tensor(
                    t3[:, 1:2 * H - 2:2, :], x3[:, :H - 1, :], 3.0 / 16.0,
                    q3[:, 1:, :], A.mult, A.add)
                # boundary rows {0,63} <- (3/16) * x rows {0,31}
                t_bnd = bass.AP(t[:].tensor, t[:].offset,
                                [t[:].ap[0], [(2 * H - 1) * W, 2], [1, W]])
                x_bnd = bass.AP(xin[:].tensor, xin[:].offset,
                                [xin[:].ap[0], [(H - 1) * W, 2], [1, W]])
                nc.scalar.mul(t_bnd, x_bnd, 3.0 / 16.0)

            o3 = o[:].rearrange("p (r c) -> p r c", c=OW)
            rows_per = OH // h_split
            for s in range(h_split):
                r0, r1 = s * rows_per, (s + 1) * rows_per
                # o[2c] = (t[c]*3) + t[c-1], c=1..31
                nc.vector.scalar_tensor_tensor(
                    o3[:, r0:r1, 2::2], t3[:, r0:r1, 1:], 3.0,
                    t3[:, r0:r1, :W - 1], A.mult, A.add)
                # o[2c+1] = (t[c]*3) + t[c+1], c=0..30
                nc.vector.scalar_tensor_tensor(
                    o3[:, r0:r1, 1:OW - 2:2], t3[:, r0:r1, :W - 1], 3.0,
                    t3[:, r0:r1, 1:], A.mult, A.add)
                # boundary cols {0,63} <- 3 * t cols {0,31}
                o_bnd = bass.AP(o[:].tensor, o[:].offset + r0 * OW,
                                [o[:].ap[0], [OW, rows_per], [OW - 1, 2]])
                t_bnd2 = bass.AP(t[:].tensor, t[:].offset + r0 * W,
                                 [t[:].ap[0], [W, rows_per], [W - 1, 2]])
                nc.scalar.mul(o_bnd, t_bnd2, 3.0)

                o_dram = bass.AP(out, base * OIMG + r0 * OW,
                                 [[OIMG, P], [1, rows_per * OW]])
                odma = out_dma
                if alt_oq and (i * h_split + s) % 2 == 1:
                    odma = nc.scalar if out_q == "sync" else nc.sync
                odma.dma_start(
                    out=o_dram,
                    in_=o[:, r0 * OW:r1 * OW])

    with tile.TileContext(nc) as tc:
        with (
            tc.tile_pool(name="pin", bufs=pin_bufs) as pin,
            tc.tile_pool(name="pxq", bufs=pxq_bufs) as pxq,
            tc.tile_pool(name="pxb", bufs=pxq_bufs) as pxb,
            tc.tile_pool(name="pt", bufs=pt_bufs) as pt,
            tc.tile_pool(name="po", bufs=po_bufs) as po,
        ):
            if loop_repeat > 1:
                with tc.For_i(0, loop_repeat):
                    for _ in range(body_reps):
                        body(tc, pin, pxq, pxb, pt, po)
            else:
                for _ in range(body_reps):
                    body(tc, pin, pxq, pxb, pt, po)
            if timing_mode:
                ts = pin.tile([1, 1], F32, tag="ts")
                in_dma.dma_start(out=ts[:],
                                 in_=bass.AP(tin, 0, [[1, 1], [1, 1]]))
                out_dma.dma_start(out=bass.AP(tout, 0, [[1, 1], [1, 1]]),
                                  in_=ts[:])
    if split_waits:
        _split_multi_waits(nc)
    return nc


def kernel(x: np.ndarray) -> np.ndarray:
    x = np.ascontiguousarray(np.asarray(x), dtype=np.float32)
    assert x.shape == (B, C, H, W), x.shape
    flat = x.reshape(IMGS, IMG)
    in_maps = [
        {"x": flat[c * PER_CORE:(c + 1) * PER_CORE]} for c in range(N_CORES)
    ]
    nc = build_nc()
    res = run_bass_kernel_spmd(nc, in_maps, core_ids=list(range(N_CORES)))
    outs = [res.results[c]["out"] for c in range(N_CORES)]
    full = np.concatenate(outs, axis=0).reshape(B, C, 2 * H, 2 * W)
    return full


if __name__ == "__main__":
    rng = np.random.default_rng(0)
    xt = rng.standard_normal((B, C, H, W), dtype=np.float32)
    yt = kernel(xt)
    print("out", yt.shape, yt.dtype)
